# revision 18
# baseline (speedup 1.0000x reference)
"""ChebGCN (K=2, 2-layer) on 8 Trainium2 NeuronCores.

Full inputs in, full output out. Internally:
  - nodes partitioned by id across 8 cores (graph-parallel, per sharding hint)
  - per-core dest nodes bin-packed into 49 blocks x 128 slots (balanced)
  - messages reduced to post-weight space first: tx1@W1 == segsum(norm * (x@W1)[col])
  - all on-device data bf16 (fp32 PSUM accumulate): fp32 matmul needs 2
    half-speed PE passes, bf16 is 1 full-speed pass
  - gather tables in HBM with TWO nodes packed per 256B row (dma_gather's
    min element size), so row ids fit int16 without a lo/hi table split;
    per-edge chunks are split by source-slot parity instead (rhs half-select)
  - dma_gather descriptor generation spread over 4 SWDGE queues
  - scatter-add via one-hot matmuls accumulating in PSUM per dest block
  - layer-2 source features exchanged with a bf16 AllGather collective
Host does sharding prep (sort/pad/index building) and output reassembly only.
"""
import sys

for _p in ("/opt/trn_rl_repo",):
    if _p not in sys.path:
        sys.path.insert(0, _p)

import numpy as np
import ml_dtypes
import concourse.bass as bass
import concourse.bacc as bacc
import concourse.mybir as mybir
import concourse.tile as tile
from concourse.bass_utils import run_bass_kernel_spmd

N = 50000
E = 800000
NCORE = 8
SH = 6250           # nodes per core
NB = 49             # dest blocks per core
P = 128
TPC = NB * P        # 6272 table slots per core
TR = NCORE * TPC    # 50176 table slots
F_IN, F_HID, F_OUT = 96, 64, 40
FP = 64             # feature dim of the message tables (one node's half-row)
G = 16              # chunks per dma_gather group (overridden per attempt)
NQ = 4              # SWDGE queues (overridden per attempt)
MB = 12             # message-group tiles in flight (gather concurrency)
PREP = True         # prepare_only+trigger_dma: decouple Pool from DMA wait

# AllGather chunking: table rows are chunk-major so each collective's
# input/output regions are contiguous; z chunks overlap layer-1 compute.
AGB = (13, 12, 12, 12)                      # blocks per AG chunk
AG0 = (0, 13, 25, 37, 49)                   # block boundaries
AGOFF = tuple(np.cumsum((0,) + tuple(8 * n * 128 for n in AGB[:-1])))


def _chunk_of_block(b):
    for j in range(len(AGB)):
        if b < AG0[j + 1]:
            return j
    raise ValueError(b)


def _gpos(core, s):
    """Table position of (core, slot) in the chunk-major layout."""
    b = s // P
    j = _chunk_of_block(b)
    return AGOFF[j] + core * AGB[j] * P + (s - AG0[j] * P)

dt = mybir.dt
bf16 = ml_dtypes.bfloat16


# ----------------------------------------------------------------- host prep
def _bin_pack_blocks(deg_local):
    order = np.argsort(-deg_local, kind="stable")
    loads = np.zeros(NB, np.int64)
    counts = np.zeros(NB, np.int32)
    slot = np.full(SH, -1, np.int64)
    big = np.iinfo(np.int64).max
    for l in order:
        b = int(np.argmin(np.where(counts < P, loads, big)))
        slot[l] = b * P + counts[b]
        counts[b] += 1
        loads[b] += deg_local[l]
    return slot


def _build_plan(edge_index):
    row = np.asarray(edge_index[0], np.int64)
    col = np.asarray(edge_index[1], np.int64)
    deg = np.bincount(row, minlength=N).astype(np.float32)
    dis = np.where(deg > 0, 1.0 / np.sqrt(np.maximum(deg, 1e-12)), 0.0).astype(np.float32)
    norm = (-dis[row] * dis[col]).astype(np.float32)

    # slot assignment: b-major (slot s = b*128 + p), table pos gs = c*TPC + s
    slot_of_node = np.zeros(N, np.int64)
    pi_inv = np.full((NCORE, TPC), -1, np.int64)
    for c in range(NCORE):
        deg_local = deg[c * SH:(c + 1) * SH].astype(np.int64)
        slot = _bin_pack_blocks(deg_local)
        slot_of_node[c * SH:(c + 1) * SH] = slot
        pi_inv[c, slot] = np.arange(c * SH, (c + 1) * SH)

    own = np.arange(N) // SH
    # chunk-major table position of each node (vectorized _gpos)
    s = slot_of_node
    b = s // P
    jj = np.digitize(b, AG0[1:])
    agoff = np.asarray(AGOFF)[jj]
    agb = np.asarray(AGB)[jj]
    ag0 = np.asarray(AG0)[jj]
    gs = agoff + own * agb * P + (s - ag0 * P)
    src_gs = gs[col]
    src_row = src_gs // 2                  # pair row (fits int16: < 25088)
    src_half = src_gs % 2                  # which half of the 256B row

    cd = row // SH                         # dest core of each edge
    dst_slot = slot_of_node[row]

    cores = []
    maxce = maxco = 0
    for c in range(NCORE):
        m = cd == c
        er = np.stack(
            [dst_slot[m], src_half[m], src_row[m],
             norm[m].view(np.int32).astype(np.int64)], axis=1)
        db = er[:, 0] // P
        er = er[np.lexsort((er[:, 2], er[:, 1], db))]
        db = er[:, 0] // P
        cores.append(er)
        for b in range(NB):
            mb = db == b
            n0 = int((er[mb, 1] == 0).sum())
            n1 = int((er[mb, 1] == 1).sum())
            maxce = max(maxce, -(-n0 // P))
            maxco = max(maxco, -(-n1 // P))
    CE, CO = max(maxce, 1), max(maxco, 1)
    CT = CE + CO
    NT = NB * CT

    def wrap_idx(v):
        n = len(v)
        a = np.zeros((16, n // 16), np.int16)
        a[np.arange(n) % 16, np.arange(n) // 16] = v
        return np.tile(a, (8, 1))

    plans = []
    for c in range(NCORE):
        er = cores[c]
        db = er[:, 0] // P
        nn = NT * P
        idx = np.zeros(nn, np.int64)
        nrm = np.zeros(nn, np.float32)
        dp = np.zeros(nn, np.int64)
        for b in range(NB):
            for h, o in ((0, b * CT * P), (1, (b * CT + CE) * P)):
                mb = (db == b) & (er[:, 1] == h)
                sub = er[mb]
                n = len(sub)
                idx[o:o + n] = sub[:, 2]
                nrm[o:o + n] = sub[:, 3].astype(np.int32).view(np.float32)
                dp[o:o + n] = sub[:, 0] % P
        plans.append({
            "idxs": wrap_idx(idx.astype(np.int16)),
            "nrm": np.ascontiguousarray(nrm.reshape(-1, P).T).astype(bf16),
            "dpt": np.ascontiguousarray(
                dp.reshape(-1, P).T.astype(np.float32)).astype(bf16),
        })

    return dict(plans=plans, pi_inv=pi_inv, CE=CE, CO=CO)


def _build_xt(x, pi_inv):
    xp = np.zeros((TR, F_IN), np.float32)
    for c in range(NCORE):
        valid = pi_inv[c] >= 0
        xp[c * TPC:(c + 1) * TPC][valid] = x[pi_inv[c][valid]]
    return np.ascontiguousarray(xp.T).astype(bf16)  # [96, TR] slot-major (cn, b, p)


# ------------------------------------------------------------------ device
def _build_graph(CE, CO):
    CT = CE + CO
    NT = NB * CT
    NG = -(-NT // G)
    nc = bacc.Bacc("TRN2", target_bir_lowering=False, num_devices=NCORE,
                   num_swdge_queues=NQ)

    f32, b16, i16 = dt.float32, dt.bfloat16, dt.int16
    xt_all = nc.dram_tensor("xt_all", [F_IN, TR], b16, kind="ExternalInput")
    xt_own = nc.dram_tensor("xt_own", [F_IN, TPC], b16, kind="ExternalInput")
    w10 = nc.dram_tensor("w10", [F_IN, F_HID], b16, kind="ExternalInput")
    w11 = nc.dram_tensor("w11", [F_IN, F_HID], b16, kind="ExternalInput")
    w20p = nc.dram_tensor("w20p", [F_HID, FP], b16, kind="ExternalInput")
    w21p = nc.dram_tensor("w21p", [F_HID, FP], b16, kind="ExternalInput")
    b1r = nc.dram_tensor("b1r", [1, F_HID], b16, kind="ExternalInput")
    b2r = nc.dram_tensor("b2r", [1, FP], b16, kind="ExternalInput")
    onesr = nc.dram_tensor("onesr", [1, P], b16, kind="ExternalInput")
    ident = nc.dram_tensor("ident", [P, P], b16, kind="ExternalInput")
    iota = nc.dram_tensor("iota", [P, P], b16, kind="ExternalInput")
    idxs = nc.dram_tensor("idxs", [P, NT * 8], i16, kind="ExternalInput")
    nrm = nc.dram_tensor("nrm", [P, NT], b16, kind="ExternalInput")
    dpt = nc.dram_tensor("dpt", [P, NT], b16, kind="ExternalInput")
    out = nc.dram_tensor("out", [P, NB, F_OUT], f32, kind="ExternalOutput")

    # message tables: [TR, 64] bf16 == [TR//2, 128] pair rows (256B each)
    y1_tab = nc.dram_tensor("y1_tab", [TR, FP], b16, kind="Internal")
    z_bounce = nc.dram_tensor("z_bounce", [TPC, FP], b16, kind="Internal")
    z_full = nc.dram_tensor("z_full", [TR, FP], b16, kind="Internal")

    with tile.TileContext(nc) as tc:
        dma_sems = [nc.alloc_semaphore(f"swdge_dma{q}") for q in range(NQ)]
        qcount = [0] * NQ
        with (
            tc.tile_pool(name="const", bufs=1) as cpool,
            tc.tile_pool(name="persist", bufs=1) as ppool,
            tc.tile_pool(name="hsp", bufs=2) as hsp,
            tc.tile_pool(name="psT", bufs=2, space="PSUM") as psT,
            tc.tile_pool(name="psZ", bufs=2, space="PSUM") as psZ,
        ):
            if PREP:
                for s in dma_sems:
                    nc.gpsimd.sem_clear(s)

            # ---- constants / persistent loads
            def load(pool, src, shape, dtype=b16, tag=None):
                t = pool.tile(shape, dtype, tag=tag)
                nc.sync.dma_start(t[:], src[:])
                return t

            w10_t = load(cpool, w10, [F_IN, F_HID], tag="w10")
            w11_t = load(cpool, w11, [F_IN, F_HID], tag="w11")
            w20_t = load(cpool, w20p, [F_HID, FP], tag="w20")
            w21_t = load(cpool, w21p, [F_HID, FP], tag="w21")
            b1_t = load(cpool, b1r, [1, F_HID], tag="b1")
            b2_t = load(cpool, b2r, [1, FP], tag="b2")
            ones_t = load(cpool, onesr, [1, P], tag="ones")
            id_t = load(cpool, ident, [P, P], tag="ident")
            io_t = load(cpool, iota, [P, P], tag="iota")
            ix_t = load(cpool, idxs, [P, NT * 8], i16, tag="ix")
            nrm_t = load(cpool, nrm, [P, NT], tag="nrm")
            dp_t = load(cpool, dpt, [P, NT], tag="dp")
            xo_t = load(ppool, xt_own, [F_IN, TPC], tag="xown")

            hT = ppool.tile([F_HID, TPC], b16, tag="hT")
            z_stages = [
                ppool.tile([P, AGB[j], FP], b16, tag=f"zst{j}", name=f"zst{j}")
                for j in range(len(AGB))
            ]
            out_stage = ppool.tile([P, NB, F_OUT], f32, tag="ost")

            # ---- phase A: y1 = x @ W1_1 for all nodes -> y1_tab (chunk-major)
            with (
                tc.tile_pool(name="xa2", bufs=2) as xa,
                tc.tile_pool(name="ya2", bufs=2) as ya,
                tc.tile_pool(name="psA", bufs=4, space="PSUM") as psA,
            ):
                BPH = min(25, NB)  # blocks per xt slice (25+24)
                for cn in range(NCORE):
                    yst = ya.tile([P, NB, FP], b16, tag="yst")
                    b0 = 0
                    eng = 0
                    for hf, nblk in ((0, BPH), (1, NB - BPH)):
                        if nblk == 0:
                            continue
                        cols = nblk * P
                        xs = xa.tile([F_IN, BPH * P], b16, tag="xs")
                        nc.sync.dma_start(
                            xs[:, :cols],
                            xt_all[:, cn * TPC + b0 * P: cn * TPC + (b0 + nblk) * P])
                        # 8 blocks share one PSUM bank; one batched eviction copy
                        for g0 in range(0, nblk, 8):
                            ng = min(8, nblk - g0)
                            ps = psA.tile([P, 8 * F_HID], f32, tag="psy")
                            for k in range(ng):
                                nc.tensor.matmul(
                                    out=ps[:, k * F_HID:(k + 1) * F_HID],
                                    lhsT=xs[:, (g0 + k) * P:(g0 + k + 1) * P],
                                    rhs=w11_t[:], start=True, stop=True)
                            dst = yst[:, b0 + g0:b0 + g0 + ng, :]
                            src = ps[:, :ng * F_HID].rearrange(
                                "p (k f) -> p k f", k=ng)
                            if eng == 0:
                                nc.vector.tensor_copy(dst, src)
                            else:
                                nc.scalar.copy(dst, src)
                            eng = (eng + 1) % 2
                        b0 += nblk
                    # chunk-major rows: region (j, cn) <- yst blocks [AG0 j..j+1)
                    for j in range(len(AGB)):
                        r0 = AGOFF[j] + cn * AGB[j] * P
                        nc.sync.dma_start(
                            y1_tab[r0:r0 + AGB[j] * P, :].rearrange(
                                "(k p) f -> p k f", p=P),
                            yst[:, AG0[j]:AG0[j + 1], :])

            # ---- spmm pass (shared for both layers)
            def spmm_pass(tab, evict, sfx, post_block=lambda b: None):
                tab_pairs = tab[:].rearrange("(r two) f -> r (two f)", two=2)
                with (
                    tc.tile_pool(name="mg" + sfx, bufs=MB) as mg,
                    tc.tile_pool(name="ohp" + sfx, bufs=3) as ohp,
                    tc.tile_pool(name="psX" + sfx, bufs=4, space="PSUM") as psX,
                ):
                    g_tiles = [None] * NG

                    def get_group(g):
                        if g_tiles[g] is not None:
                            return g_tiles[g]
                        ncg = min(G, NT - g * G)
                        ni = ncg * P
                        q = g % NQ
                        m = mg.tile([P, G, 2 * FP], b16, tag="mg")
                        if PREP:
                            nc.gpsimd.dma_gather(
                                m[:, :ncg, :], tab_pairs,
                                ix_t[:, g * G * 8:(g * G + ncg) * 8],
                                ni, ni, 2 * FP, single_packet=False,
                                queue_num=q, prepare_only=True,
                                sem=dma_sems[q])
                            nc.gpsimd.trigger_dma(count=None, queue_num=q)
                            qcount[q] += 1
                            nc.vector.wait_ge(dma_sems[q], 16 * qcount[q])
                        else:
                            nc.gpsimd.dma_gather(
                                m[:, :ncg, :], tab_pairs,
                                ix_t[:, g * G * 8:(g * G + ncg) * 8],
                                ni, ni, 2 * FP, single_packet=False,
                                queue_num=q)
                        nc.vector.tensor_tensor(
                            out=m[:, :ncg, :],
                            in0=m[:, :ncg, :],
                            in1=nrm_t[:, g * G:g * G + ncg].to_broadcast(
                                [P, ncg, 2 * FP]),
                            op=mybir.AluOpType.mult)
                        g_tiles[g] = m
                        return m

                    for b in range(NB):
                        ps = psX.tile([P, FP], f32, tag="acc")
                        oh = ohp.tile([P, CT * P], b16, tag="oh")
                        nc.vector.tensor_tensor(
                            out=oh[:].rearrange("p (c j) -> p c j", c=CT),
                            in0=dp_t[:, b * CT:(b + 1) * CT].to_broadcast(
                                [P, CT, P]),
                            in1=bass.AP(io_t[:].tensor, io_t[:].offset,
                                        [io_t[:].ap[0], [0, CT], [1, P]]),
                            op=mybir.AluOpType.is_equal)
                        for j in range(CT):
                            q = b * CT + j
                            m = get_group(q // G)
                            h = 0 if j < CE else 1
                            nc.tensor.matmul(
                                out=ps[:], lhsT=oh[:, j * P:(j + 1) * P],
                                rhs=m[:, q % G, h * FP:(h + 1) * FP],
                                start=(j == 0), stop=False)
                        evict(b, ps)
                        post_block(b)

            # ---- layer 1 eviction: h block
            def evict_l1(b, ps):
                nc.tensor.matmul(out=ps[:], lhsT=xo_t[:, b * P:(b + 1) * P],
                                 rhs=w10_t[:], start=False, stop=False)
                nc.tensor.matmul(out=ps[:], lhsT=ones_t[:], rhs=b1_t[:],
                                 start=False, stop=True)
                hs = hsp.tile([P, F_HID], b16, tag="hs")
                nc.scalar.activation(hs[:], ps[:], mybir.ActivationFunctionType.Relu)
                pt = psT.tile([F_HID, P], b16, tag="pt")
                nc.tensor.transpose(out=pt[:], in_=hs[:], identity=id_t[:])
                nc.vector.tensor_copy(hT[:, b * P:(b + 1) * P], pt[:])
                pz = psZ.tile([P, FP], f32, tag="pz")
                nc.tensor.matmul(out=pz[:], lhsT=hT[:, b * P:(b + 1) * P],
                                 rhs=w21_t[:], start=True, stop=True)
                j = _chunk_of_block(b)
                nc.scalar.copy(z_stages[j][:, b - AG0[j], :], pz[:])

            # fire each z AllGather chunk as soon as its last block is done,
            # overlapping the collective with remaining layer-1 compute
            def post_l1(b):
                for j in range(len(AGB)):
                    if b != AG0[j + 1] - 1:
                        continue
                    r0 = AG0[j] * P
                    nc.sync.dma_start(
                        z_bounce[r0:r0 + AGB[j] * P, :].rearrange(
                            "(k p) f -> p k f", p=P),
                        z_stages[j][:])
                    nc.gpsimd.collective_compute(
                        "AllGather", mybir.AluOpType.bypass,
                        replica_groups=[list(range(NCORE))],
                        ins=[z_bounce[r0:r0 + AGB[j] * P, :].opt()],
                        outs=[z_full[AGOFF[j]:AGOFF[j] + 8 * AGB[j] * P, :].opt()],
                    )

            spmm_pass(y1_tab, evict_l1, "a", post_l1)

            # ---- layer 2 eviction: out block
            def evict_l2(b, ps):
                nc.tensor.matmul(out=ps[:], lhsT=hT[:, b * P:(b + 1) * P],
                                 rhs=w20_t[:], start=False, stop=False)
                nc.tensor.matmul(out=ps[:], lhsT=ones_t[:], rhs=b2_t[:],
                                 start=False, stop=True)
                if b % 2 == 0:
                    nc.scalar.copy(out_stage[:, b, :], ps[:, :F_OUT])
                else:
                    nc.vector.tensor_copy(out_stage[:, b, :], ps[:, :F_OUT])

            spmm_pass(z_full, evict_l2, "b")

            nc.sync.dma_start(out[:], out_stage[:])

    nc.compile()
    return nc


_GRAPH_CACHE = {}


def kernel(x, edge_index, W1_0, W1_1, b1, W2_0, W2_1, b2):
    x = np.asarray(x, np.float32)
    W1_0 = np.asarray(W1_0, np.float32)
    W1_1 = np.asarray(W1_1, np.float32)
    b1 = np.asarray(b1, np.float32)
    W2_0 = np.asarray(W2_0, np.float32)
    W2_1 = np.asarray(W2_1, np.float32)
    b2 = np.asarray(b2, np.float32)

    plan = _build_plan(edge_index)
    CE, CO = plan["CE"], plan["CO"]

    xt = _build_xt(x, plan["pi_inv"])
    w20p = np.zeros((F_HID, FP), np.float32); w20p[:, :F_OUT] = W2_0
    w21p = np.zeros((F_HID, FP), np.float32); w21p[:, :F_OUT] = W2_1
    b2p = np.zeros((1, FP), np.float32); b2p[0, :F_OUT] = b2
    ident = np.eye(P, dtype=np.float32)
    iota = np.tile(np.arange(P, dtype=np.float32), (P, 1))
    ones = np.ones((1, P), np.float32)

    common = dict(
        xt_all=xt,
        w10=W1_0.astype(bf16), w11=W1_1.astype(bf16),
        w20p=w20p.astype(bf16), w21p=w21p.astype(bf16),
        b1r=b1.reshape(1, F_HID).astype(bf16), b2r=b2p.astype(bf16),
        onesr=ones.astype(bf16), ident=ident.astype(bf16),
        iota=iota.astype(bf16),
    )
    in_maps = []
    for c in range(NCORE):
        pl = plan["plans"][c]
        m = dict(common)
        m["xt_own"] = np.ascontiguousarray(xt[:, c * TPC:(c + 1) * TPC])
        m["idxs"] = pl["idxs"]
        m["nrm"] = pl["nrm"]
        m["dpt"] = pl["dpt"]
        in_maps.append(m)

    global G, NQ, PREP
    res = None
    last_exc = None
    for g_try, nq_try, prep_try in (
            (16, 4, True), (16, 4, False), (16, 2, False), (4, 1, False)):
        G, NQ, PREP = g_try, nq_try, prep_try
        key = (CE, CO, g_try, nq_try, prep_try)
        try:
            if key not in _GRAPH_CACHE:
                _GRAPH_CACHE[key] = _build_graph(CE, CO)
            res = run_bass_kernel_spmd(
                _GRAPH_CACHE[key], in_maps, core_ids=list(range(NCORE)))
            break
        except Exception as e:  # noqa: BLE001 - retry with safer gather size
            last_exc = e
            import time as _t
            _t.sleep(10)
    if res is None:
        raise last_exc
    kernel.last_result = res

    out_full = np.zeros((N, F_OUT), np.float32)
    pi_inv = plan["pi_inv"]
    for c in range(NCORE):
        o = res.results[c]["out"].transpose(1, 0, 2).reshape(TPC, F_OUT)
        valid = pi_inv[c] >= 0
        out_full[pi_inv[c][valid]] = o[valid]
    return out_full


# revision 34
# speedup vs baseline: 1.7312x; 1.7312x over previous
"""ChebGCN (K=2, 2-layer) on 8 Trainium2 NeuronCores.

Full inputs in, full output out. Internally:
  - nodes partitioned by id across 8 cores (graph-parallel, per sharding hint)
  - per-core dest nodes bin-packed into 49 blocks x 128 slots (balanced)
  - messages reduced to post-weight space first: tx1@W1 == segsum(norm * (x@W1)[col])
  - all on-device data bf16 (fp32 PSUM accumulate): fp32 matmul needs 2
    half-speed PE passes, bf16 is 1 full-speed pass
  - gather tables in HBM with TWO nodes packed per 256B row (dma_gather's
    min element size), so row ids fit int16 without a lo/hi table split;
    per-edge chunks are split by source-slot parity instead (rhs half-select)
  - dma_gather descriptor generation spread over 4 SWDGE queues
  - scatter-add via one-hot matmuls accumulating in PSUM per dest block
  - layer-2 source features exchanged with a bf16 AllGather collective
Host does sharding prep (sort/pad/index building) and output reassembly only.
"""
import sys

for _p in ("/opt/trn_rl_repo",):
    if _p not in sys.path:
        sys.path.insert(0, _p)

import numpy as np
import ml_dtypes
import concourse.bass as bass
import concourse.bacc as bacc
import concourse.mybir as mybir
import concourse.tile as tile
from concourse.bass_utils import run_bass_kernel_spmd

N = 50000
E = 800000
NCORE = 8
SH = 6250           # nodes per core
NB = 49             # dest blocks per core
P = 128
TPC = NB * P        # 6272 table slots per core
TR = NCORE * TPC    # 50176 table slots
F_IN, F_HID, F_OUT = 96, 64, 40
FP = 64             # feature dim of the message tables (one node's half-row)
G = 16              # chunks per dma_gather group (overridden per attempt)
NQ = 4              # SWDGE queues (overridden per attempt)
MB = 16             # message-group tiles in flight (gather concurrency)
PREP = False        # prepare_only+trigger_dma (slower on this ucode: gen
                    # cost dominates and the prep path pays ~8ns/desc)

# AllGather chunking: table rows are chunk-major so each collective's
# input/output regions are contiguous; z chunks overlap layer-1 compute.
AGB = (13, 12, 12, 12)                      # blocks per AG chunk
AG0 = (0, 13, 25, 37, 49)                   # block boundaries
AGOFF = tuple(np.cumsum((0,) + tuple(8 * n * 128 for n in AGB[:-1])))


def _chunk_of_block(b):
    for j in range(len(AGB)):
        if b < AG0[j + 1]:
            return j
    raise ValueError(b)


def _gpos(core, s):
    """Table position of (core, slot) in the chunk-major layout."""
    b = s // P
    j = _chunk_of_block(b)
    return AGOFF[j] + core * AGB[j] * P + (s - AG0[j] * P)

dt = mybir.dt
bf16 = ml_dtypes.bfloat16


# ----------------------------------------------------------------- host prep
def _bin_pack_blocks(deg_even, deg_odd):
    """Assign local nodes to blocks balancing even- and odd-half in-degree
    jointly (the per-block parity maxima set the chunk counts CE/CO).

    Every node's table-position parity is pre-committed to its local index
    parity (l % 2), so each block has 64 even and 64 odd slots; edge halves
    (source parity) are then fixed regardless of the packing.
    """
    tot = deg_even + deg_odd
    order = np.argsort(-tot, kind="stable")
    le = np.zeros(NB, np.int64)
    lo = np.zeros(NB, np.int64)
    cnt = np.zeros((NB, 2), np.int32)   # filled slots per parity class
    slot = np.full(SH, -1, np.int64)
    big = np.float64(1e18)
    for l in order:
        par = l % 2
        e, o = deg_even[l], deg_odd[l]
        cost = np.where(cnt[:, par] < P // 2,
                        np.maximum(le + e, lo + o)
                        + 1e-3 * (le + e + lo + o), big)
        b = int(np.argmin(cost))
        slot[l] = b * P + 2 * cnt[b, par] + par
        cnt[b, par] += 1
        le[b] += e
        lo[b] += o
    return slot


def _build_plan(edge_index):
    row = np.asarray(edge_index[0], np.int64)
    col = np.asarray(edge_index[1], np.int64)
    deg = np.bincount(row, minlength=N).astype(np.float32)
    dis = np.where(deg > 0, 1.0 / np.sqrt(np.maximum(deg, 1e-12)), 0.0).astype(np.float32)
    norm = (-dis[row] * dis[col]).astype(np.float32)

    # slot assignment: b-major (slot s = b*128 + p), table pos gs = c*TPC + s
    # per-node in-degree split by source parity (col % 2 == source slot parity)
    src_par = (col % 2).astype(np.int64)
    deg_e = np.bincount(row[src_par == 0], minlength=N).astype(np.int64)
    deg_o = np.bincount(row[src_par == 1], minlength=N).astype(np.int64)
    slot_of_node = np.zeros(N, np.int64)
    pi_inv = np.full((NCORE, TPC), -1, np.int64)
    for c in range(NCORE):
        slot = _bin_pack_blocks(deg_e[c * SH:(c + 1) * SH],
                                deg_o[c * SH:(c + 1) * SH])
        slot_of_node[c * SH:(c + 1) * SH] = slot
        pi_inv[c, slot] = np.arange(c * SH, (c + 1) * SH)

    own = np.arange(N) // SH
    # chunk-major table position of each node (vectorized _gpos)
    s = slot_of_node
    b = s // P
    jj = np.digitize(b, AG0[1:])
    agoff = np.asarray(AGOFF)[jj]
    agb = np.asarray(AGB)[jj]
    ag0 = np.asarray(AG0)[jj]
    gs = agoff + own * agb * P + (s - ag0 * P)
    src_gs = gs[col]
    src_row = src_gs // 2                  # pair row (fits int16: < 25088)
    src_half = src_gs % 2                  # which half of the 256B row

    cd = row // SH                         # dest core of each edge
    dst_slot = slot_of_node[row]

    cores = []
    maxce = maxco = 0
    for c in range(NCORE):
        m = cd == c
        er = np.stack(
            [dst_slot[m], src_half[m], src_row[m],
             norm[m].view(np.int32).astype(np.int64)], axis=1)
        db = er[:, 0] // P
        er = er[np.lexsort((er[:, 2], er[:, 1], db))]
        db = er[:, 0] // P
        cores.append(er)
        for b in range(NB):
            mb = db == b
            n0 = int((er[mb, 1] == 0).sum())
            n1 = int((er[mb, 1] == 1).sum())
            maxce = max(maxce, -(-n0 // P))
            maxco = max(maxco, -(-n1 // P))
    CE, CO = max(maxce, 1), max(maxco, 1)
    CT = CE + CO
    NT = NB * CT

    def wrap_idx(v):
        n = len(v)
        a = np.zeros((16, n // 16), np.int16)
        a[np.arange(n) % 16, np.arange(n) // 16] = v
        return np.tile(a, (8, 1))

    plans = []
    for c in range(NCORE):
        er = cores[c]
        db = er[:, 0] // P
        nn = NT * P
        idx = np.zeros(nn, np.int64)
        nrm = np.zeros(nn, np.float32)
        dp = np.zeros(nn, np.int64)
        for b in range(NB):
            for h, o in ((0, b * CT * P), (1, (b * CT + CE) * P)):
                mb = (db == b) & (er[:, 1] == h)
                sub = er[mb]
                n = len(sub)
                idx[o:o + n] = sub[:, 2]
                nrm[o:o + n] = sub[:, 3].astype(np.int32).view(np.float32)
                dp[o:o + n] = sub[:, 0] % P
        plans.append({
            "idxs": wrap_idx(idx.astype(np.int16)),
            "nrm": np.ascontiguousarray(nrm.reshape(-1, P).T).astype(bf16),
            "dpt": np.ascontiguousarray(
                dp.reshape(-1, P).T.astype(np.float32)).astype(bf16),
        })

    return dict(plans=plans, pi_inv=pi_inv, CE=CE, CO=CO)


def _build_xt(x, pi_inv):
    xp = np.zeros((TR, F_IN), np.float32)
    for c in range(NCORE):
        valid = pi_inv[c] >= 0
        xp[c * TPC:(c + 1) * TPC][valid] = x[pi_inv[c][valid]]
    return np.ascontiguousarray(xp.T).astype(bf16)  # [96, TR] slot-major (cn, b, p)


# ------------------------------------------------------------------ device
def _build_graph(CE, CO):
    CT = CE + CO
    NT = NB * CT
    NG = -(-NT // G)
    nc = bacc.Bacc("TRN2", target_bir_lowering=False, num_devices=NCORE,
                   num_swdge_queues=NQ)

    f32, b16, i16 = dt.float32, dt.bfloat16, dt.int16
    xt_all = nc.dram_tensor("xt_all", [F_IN, TR], b16, kind="ExternalInput")
    xt_own = nc.dram_tensor("xt_own", [F_IN, TPC], b16, kind="ExternalInput")
    w10 = nc.dram_tensor("w10", [F_IN, F_HID], b16, kind="ExternalInput")
    w11 = nc.dram_tensor("w11", [F_IN, F_HID], b16, kind="ExternalInput")
    w20p = nc.dram_tensor("w20p", [F_HID, FP], b16, kind="ExternalInput")
    w21p = nc.dram_tensor("w21p", [F_HID, FP], b16, kind="ExternalInput")
    b1r = nc.dram_tensor("b1r", [1, F_HID], b16, kind="ExternalInput")
    b2r = nc.dram_tensor("b2r", [1, FP], b16, kind="ExternalInput")
    onesr = nc.dram_tensor("onesr", [1, P], b16, kind="ExternalInput")
    ident = nc.dram_tensor("ident", [P, P], b16, kind="ExternalInput")
    iota = nc.dram_tensor("iota", [P, P], b16, kind="ExternalInput")
    idxs = nc.dram_tensor("idxs", [P, NT * 8], i16, kind="ExternalInput")
    nrm = nc.dram_tensor("nrm", [P, NT], b16, kind="ExternalInput")
    dpt = nc.dram_tensor("dpt", [P, NT], b16, kind="ExternalInput")
    out = nc.dram_tensor("out", [P, NB, F_OUT], f32, kind="ExternalOutput")

    # message tables: [TR, 64] bf16 == [TR//2, 128] pair rows (256B each)
    y1_tab = nc.dram_tensor("y1_tab", [TR, FP], b16, kind="Internal")
    z_bounce = nc.dram_tensor("z_bounce", [TPC, FP], b16, kind="Internal")
    z_full = nc.dram_tensor("z_full", [TR, FP], b16, kind="Internal")

    with tile.TileContext(nc) as tc:
        dma_sems = [nc.alloc_semaphore(f"swdge_dma{q}") for q in range(NQ)]
        qcount = [0] * NQ
        with (
            tc.tile_pool(name="const", bufs=1) as cpool,
            tc.tile_pool(name="persist", bufs=1) as ppool,
            tc.tile_pool(name="hsp", bufs=4) as hsp,
            # gather-message and one-hot pools are shared by both spmm
            # passes: a per-pass scoped pool would reuse the same SBUF stack
            # region, serializing layer 2's first gathers against ALL of
            # layer 1's consumers (~100us hole at the layer boundary).
            tc.tile_pool(name="mgs", bufs=MB) as mgp,
            tc.tile_pool(name="ohps", bufs=6) as ohpp,
            tc.tile_pool(name="psT", bufs=2, space="PSUM") as psT,
            tc.tile_pool(name="psZ", bufs=2, space="PSUM") as psZ,
        ):
            if PREP:
                for s in dma_sems:
                    nc.gpsimd.sem_clear(s)

            # ---- constants / persistent loads
            def load(pool, src, shape, dtype=b16, tag=None):
                t = pool.tile(shape, dtype, tag=tag)
                nc.sync.dma_start(t[:], src[:])
                return t

            w10_t = load(cpool, w10, [F_IN, F_HID], tag="w10")
            w11_t = load(cpool, w11, [F_IN, F_HID], tag="w11")
            w20_t = load(cpool, w20p, [F_HID, FP], tag="w20")
            w21_t = load(cpool, w21p, [F_HID, FP], tag="w21")
            b1_t = load(cpool, b1r, [1, F_HID], tag="b1")
            b2_t = load(cpool, b2r, [1, FP], tag="b2")
            ones_t = load(cpool, onesr, [1, P], tag="ones")
            id_t = load(cpool, ident, [P, P], tag="ident")
            io_t = load(cpool, iota, [P, P], tag="iota")
            ix_t = load(cpool, idxs, [P, NT * 8], i16, tag="ix")
            nrm_t = load(cpool, nrm, [P, NT], tag="nrm")
            dp_t = load(cpool, dpt, [P, NT], tag="dp")
            xo_t = load(ppool, xt_own, [F_IN, TPC], tag="xown")

            hT = ppool.tile([F_HID, TPC], b16, tag="hT")
            z_stages = [
                ppool.tile([P, AGB[j], FP], b16, tag=f"zst{j}", name=f"zst{j}")
                for j in range(len(AGB))
            ]
            out_stage = ppool.tile([P, NB, F_OUT], f32, tag="ost")

            # ---- phase A: y1 = x @ W1_1 for all nodes -> y1_tab (chunk-major)
            with (
                tc.tile_pool(name="xa2", bufs=2) as xa,
                tc.tile_pool(name="ya2", bufs=2) as ya,
                tc.tile_pool(name="psA", bufs=4, space="PSUM") as psA,
            ):
                BPH = min(25, NB)  # blocks per xt slice (25+24)
                for cn in range(NCORE):
                    yst = ya.tile([P, NB, FP], b16, tag="yst")
                    b0 = 0
                    eng = 0
                    for hf, nblk in ((0, BPH), (1, NB - BPH)):
                        if nblk == 0:
                            continue
                        cols = nblk * P
                        xs = xa.tile([F_IN, BPH * P], b16, tag="xs")
                        nc.sync.dma_start(
                            xs[:, :cols],
                            xt_all[:, cn * TPC + b0 * P: cn * TPC + (b0 + nblk) * P])
                        # 8 blocks share one PSUM bank; one batched eviction copy
                        for g0 in range(0, nblk, 8):
                            ng = min(8, nblk - g0)
                            ps = psA.tile([P, 8 * F_HID], f32, tag="psy")
                            for k in range(ng):
                                nc.tensor.matmul(
                                    out=ps[:, k * F_HID:(k + 1) * F_HID],
                                    lhsT=xs[:, (g0 + k) * P:(g0 + k + 1) * P],
                                    rhs=w11_t[:], start=True, stop=True)
                            dst = yst[:, b0 + g0:b0 + g0 + ng, :]
                            src = ps[:, :ng * F_HID].rearrange(
                                "p (k f) -> p k f", k=ng)
                            if eng == 0:
                                nc.vector.tensor_copy(dst, src)
                            else:
                                nc.scalar.copy(dst, src)
                            eng = (eng + 1) % 2
                        b0 += nblk
                    # chunk-major rows: region (j, cn) <- yst blocks [AG0 j..j+1)
                    for j in range(len(AGB)):
                        r0 = AGOFF[j] + cn * AGB[j] * P
                        nc.sync.dma_start(
                            y1_tab[r0:r0 + AGB[j] * P, :].rearrange(
                                "(k p) f -> p k f", p=P),
                            yst[:, AG0[j]:AG0[j + 1], :])

            # ---- spmm pass (shared for both layers)
            def spmm_pass(tab, evict, sfx, post_block=lambda b: None):
                tab_pairs = tab[:].rearrange("(r two) f -> r (two f)", two=2)
                mg, ohp = mgp, ohpp
                with (
                    tc.tile_pool(name="psX" + sfx, bufs=4, space="PSUM") as psX,
                ):
                    g_tiles = [None] * NG

                    def get_group(g):
                        if g_tiles[g] is not None:
                            return g_tiles[g]
                        ncg = min(G, NT - g * G)
                        ni = ncg * P
                        q = g % NQ
                        m = mg.tile([P, G, 2 * FP], b16, tag="mg")
                        if PREP:
                            nc.gpsimd.dma_gather(
                                m[:, :ncg, :], tab_pairs,
                                ix_t[:, g * G * 8:(g * G + ncg) * 8],
                                ni, ni, 2 * FP, single_packet=False,
                                queue_num=q, prepare_only=True,
                                sem=dma_sems[q])
                            nc.gpsimd.trigger_dma(count=None, queue_num=q)
                            qcount[q] += 1
                            nc.vector.wait_ge(dma_sems[q], 16 * qcount[q])
                        else:
                            nc.gpsimd.dma_gather(
                                m[:, :ncg, :], tab_pairs,
                                ix_t[:, g * G * 8:(g * G + ncg) * 8],
                                ni, ni, 2 * FP, single_packet=False,
                                queue_num=q)
                        nc.vector.tensor_tensor(
                            out=m[:, :ncg, :],
                            in0=m[:, :ncg, :],
                            in1=nrm_t[:, g * G:g * G + ncg].to_broadcast(
                                [P, ncg, 2 * FP]),
                            op=mybir.AluOpType.mult)
                        g_tiles[g] = m
                        return m

                    for b in range(NB):
                        ps = psX.tile([P, FP], f32, tag="acc")
                        oh = ohp.tile([P, CT * P], b16, tag="oh")
                        nc.vector.tensor_tensor(
                            out=oh[:].rearrange("p (c j) -> p c j", c=CT),
                            in0=dp_t[:, b * CT:(b + 1) * CT].to_broadcast(
                                [P, CT, P]),
                            in1=bass.AP(io_t[:].tensor, io_t[:].offset,
                                        [io_t[:].ap[0], [0, CT], [1, P]]),
                            op=mybir.AluOpType.is_equal)
                        for j in range(CT):
                            q = b * CT + j
                            m = get_group(q // G)
                            h = 0 if j < CE else 1
                            nc.tensor.matmul(
                                out=ps[:], lhsT=oh[:, j * P:(j + 1) * P],
                                rhs=m[:, q % G, h * FP:(h + 1) * FP],
                                start=(j == 0), stop=False)
                        evict(b, ps)
                        post_block(b)

            # ---- layer 1 eviction: h block
            def evict_l1(b, ps):
                nc.tensor.matmul(out=ps[:], lhsT=xo_t[:, b * P:(b + 1) * P],
                                 rhs=w10_t[:], start=False, stop=False)
                nc.tensor.matmul(out=ps[:], lhsT=ones_t[:], rhs=b1_t[:],
                                 start=False, stop=True)
                hs = hsp.tile([P, F_HID], b16, tag="hs")
                nc.scalar.activation(hs[:], ps[:], mybir.ActivationFunctionType.Relu)
                pt = psT.tile([F_HID, P], b16, tag="pt")
                nc.tensor.transpose(out=pt[:], in_=hs[:], identity=id_t[:])
                nc.vector.tensor_copy(hT[:, b * P:(b + 1) * P], pt[:])
                pz = psZ.tile([P, FP], f32, tag="pz")
                nc.tensor.matmul(out=pz[:], lhsT=hT[:, b * P:(b + 1) * P],
                                 rhs=w21_t[:], start=True, stop=True)
                j = _chunk_of_block(b)
                nc.scalar.copy(z_stages[j][:, b - AG0[j], :], pz[:])

            # Stage each z chunk to HBM as soon as its last block is evicted,
            # but delay the AllGather *emission* ~10 blocks: the collective is
            # issued on the in-order Pool engine, and Pool runs ~14 blocks of
            # gather issue ahead of the evictions — emitting the AG at the
            # data-ready block stalls the whole gather chain behind its wait.
            AGDELAY = 10

            def post_l1(b):
                for j in range(len(AGB)):
                    if b == AG0[j + 1] - 1:
                        r0 = AG0[j] * P
                        nc.sync.dma_start(
                            z_bounce[r0:r0 + AGB[j] * P, :].rearrange(
                                "(k p) f -> p k f", p=P),
                            z_stages[j][:])
                    if b == min(AG0[j + 1] - 1 + AGDELAY, NB - 1):
                        r0 = AG0[j] * P
                        nc.gpsimd.collective_compute(
                            "AllGather", mybir.AluOpType.bypass,
                            replica_groups=[list(range(NCORE))],
                            ins=[z_bounce[r0:r0 + AGB[j] * P, :].opt()],
                            outs=[z_full[AGOFF[j]:
                                         AGOFF[j] + 8 * AGB[j] * P, :].opt()],
                        )

            spmm_pass(y1_tab, evict_l1, "a", post_l1)

            # ---- layer 2 eviction: out block
            def evict_l2(b, ps):
                nc.tensor.matmul(out=ps[:], lhsT=hT[:, b * P:(b + 1) * P],
                                 rhs=w20_t[:], start=False, stop=False)
                nc.tensor.matmul(out=ps[:], lhsT=ones_t[:], rhs=b2_t[:],
                                 start=False, stop=True)
                if b % 2 == 0:
                    nc.scalar.copy(out_stage[:, b, :], ps[:, :F_OUT])
                else:
                    nc.vector.tensor_copy(out_stage[:, b, :], ps[:, :F_OUT])

            spmm_pass(z_full, evict_l2, "b")

            nc.sync.dma_start(out[:], out_stage[:])

    nc.compile()
    return nc


_GRAPH_CACHE = {}


def kernel(x, edge_index, W1_0, W1_1, b1, W2_0, W2_1, b2):
    x = np.asarray(x, np.float32)
    W1_0 = np.asarray(W1_0, np.float32)
    W1_1 = np.asarray(W1_1, np.float32)
    b1 = np.asarray(b1, np.float32)
    W2_0 = np.asarray(W2_0, np.float32)
    W2_1 = np.asarray(W2_1, np.float32)
    b2 = np.asarray(b2, np.float32)

    plan = _build_plan(edge_index)
    CE, CO = plan["CE"], plan["CO"]

    xt = _build_xt(x, plan["pi_inv"])
    w20p = np.zeros((F_HID, FP), np.float32); w20p[:, :F_OUT] = W2_0
    w21p = np.zeros((F_HID, FP), np.float32); w21p[:, :F_OUT] = W2_1
    b2p = np.zeros((1, FP), np.float32); b2p[0, :F_OUT] = b2
    ident = np.eye(P, dtype=np.float32)
    iota = np.tile(np.arange(P, dtype=np.float32), (P, 1))
    ones = np.ones((1, P), np.float32)

    common = dict(
        xt_all=xt,
        w10=W1_0.astype(bf16), w11=W1_1.astype(bf16),
        w20p=w20p.astype(bf16), w21p=w21p.astype(bf16),
        b1r=b1.reshape(1, F_HID).astype(bf16), b2r=b2p.astype(bf16),
        onesr=ones.astype(bf16), ident=ident.astype(bf16),
        iota=iota.astype(bf16),
    )
    in_maps = []
    for c in range(NCORE):
        pl = plan["plans"][c]
        m = dict(common)
        m["xt_own"] = np.ascontiguousarray(xt[:, c * TPC:(c + 1) * TPC])
        m["idxs"] = pl["idxs"]
        m["nrm"] = pl["nrm"]
        m["dpt"] = pl["dpt"]
        in_maps.append(m)

    global G, NQ, PREP
    res = None
    last_exc = None
    for g_try, nq_try, prep_try in (
            (16, 4, False), (16, 2, False), (4, 1, False)):
        G, NQ, PREP = g_try, nq_try, prep_try
        key = (CE, CO, g_try, nq_try, prep_try)
        try:
            if key not in _GRAPH_CACHE:
                _GRAPH_CACHE[key] = _build_graph(CE, CO)
            res = run_bass_kernel_spmd(
                _GRAPH_CACHE[key], in_maps, core_ids=list(range(NCORE)))
            break
        except Exception as e:  # noqa: BLE001 - retry with safer gather size
            last_exc = e
            import time as _t
            _t.sleep(10)
    if res is None:
        raise last_exc
    kernel.last_result = res

    out_full = np.zeros((N, F_OUT), np.float32)
    pi_inv = plan["pi_inv"]
    for c in range(NCORE):
        o = res.results[c]["out"].transpose(1, 0, 2).reshape(TPC, F_OUT)
        valid = pi_inv[c] >= 0
        out_full[pi_inv[c][valid]] = o[valid]
    return out_full


# revision 37
# speedup vs baseline: 1.7721x; 1.0236x over previous
"""ChebGCN (K=2, 2-layer) on 8 Trainium2 NeuronCores.

Full inputs in, full output out. Internally:
  - nodes partitioned by id across 8 cores (graph-parallel, per sharding hint)
  - per-core dest nodes bin-packed into 49 blocks x 128 slots (balanced)
  - messages reduced to post-weight space first: tx1@W1 == segsum(norm * (x@W1)[col])
  - all on-device data bf16 (fp32 PSUM accumulate): fp32 matmul needs 2
    half-speed PE passes, bf16 is 1 full-speed pass
  - gather tables in HBM with TWO nodes packed per 256B row (dma_gather's
    min element size), so row ids fit int16 without a lo/hi table split;
    per-edge chunks are split by source-slot parity instead (rhs half-select)
  - dma_gather descriptor generation spread over 4 SWDGE queues
  - scatter-add via one-hot matmuls accumulating in PSUM per dest block
  - layer-2 source features exchanged with a bf16 AllGather collective
Host does sharding prep (sort/pad/index building) and output reassembly only.
"""
import sys

for _p in ("/opt/trn_rl_repo",):
    if _p not in sys.path:
        sys.path.insert(0, _p)

import numpy as np
import ml_dtypes
import concourse.bass as bass
import concourse.bacc as bacc
import concourse.mybir as mybir
import concourse.tile as tile
from concourse.bass_utils import run_bass_kernel_spmd

N = 50000
E = 800000
NCORE = 8
SH = 6250           # nodes per core
NB = 49             # dest blocks per core
P = 128
TPC = NB * P        # 6272 table slots per core
TR = NCORE * TPC    # 50176 table slots
F_IN, F_HID, F_OUT = 96, 64, 40
FP = 64             # feature dim of the message tables (one node's half-row)
G = 16              # chunks per dma_gather group (overridden per attempt)
NQ = 4              # SWDGE queues (overridden per attempt)
MB = 16             # message-group tiles in flight (gather concurrency)
PREP = False        # prepare_only+trigger_dma (slower on this ucode: gen
                    # cost dominates and the prep path pays ~8ns/desc)

# AllGather chunking: table rows are chunk-major so each collective's
# input/output regions are contiguous; z chunks overlap layer-1 compute.
AGB = (13, 12, 12, 12)                      # blocks per AG chunk
AG0 = (0, 13, 25, 37, 49)                   # block boundaries
AGOFF = tuple(np.cumsum((0,) + tuple(8 * n * 128 for n in AGB[:-1])))


def _chunk_of_block(b):
    for j in range(len(AGB)):
        if b < AG0[j + 1]:
            return j
    raise ValueError(b)


def _gpos(core, s):
    """Table position of (core, slot) in the chunk-major layout."""
    b = s // P
    j = _chunk_of_block(b)
    return AGOFF[j] + core * AGB[j] * P + (s - AG0[j] * P)

dt = mybir.dt
bf16 = ml_dtypes.bfloat16


# ----------------------------------------------------------------- host prep
def _bin_pack_blocks(deg_even, deg_odd):
    """Assign local nodes to blocks balancing even- and odd-half in-degree
    jointly (the per-block parity maxima set the chunk counts CE/CO).

    Every node's table-position parity is pre-committed to its local index
    parity (l % 2), so each block has 64 even and 64 odd slots; edge halves
    (source parity) are then fixed regardless of the packing.
    """
    tot = deg_even + deg_odd
    order = np.argsort(-tot, kind="stable")
    le = np.zeros(NB, np.int64)
    lo = np.zeros(NB, np.int64)
    cnt = np.zeros((NB, 2), np.int32)   # filled slots per parity class
    slot = np.full(SH, -1, np.int64)
    big = np.float64(1e18)
    for l in order:
        par = l % 2
        e, o = deg_even[l], deg_odd[l]
        cost = np.where(cnt[:, par] < P // 2,
                        np.maximum(le + e, lo + o)
                        + 1e-3 * (le + e + lo + o), big)
        b = int(np.argmin(cost))
        slot[l] = b * P + 2 * cnt[b, par] + par
        cnt[b, par] += 1
        le[b] += e
        lo[b] += o
    return slot


def _build_plan(edge_index):
    row = np.asarray(edge_index[0], np.int64)
    col = np.asarray(edge_index[1], np.int64)
    deg = np.bincount(row, minlength=N).astype(np.float32)
    dis = np.where(deg > 0, 1.0 / np.sqrt(np.maximum(deg, 1e-12)), 0.0).astype(np.float32)
    norm = (-dis[row] * dis[col]).astype(np.float32)

    # slot assignment: b-major (slot s = b*128 + p), table pos gs = c*TPC + s
    # per-node in-degree split by source parity (col % 2 == source slot parity)
    src_par = (col % 2).astype(np.int64)
    deg_e = np.bincount(row[src_par == 0], minlength=N).astype(np.int64)
    deg_o = np.bincount(row[src_par == 1], minlength=N).astype(np.int64)
    slot_of_node = np.zeros(N, np.int64)
    pi_inv = np.full((NCORE, TPC), -1, np.int64)
    for c in range(NCORE):
        slot = _bin_pack_blocks(deg_e[c * SH:(c + 1) * SH],
                                deg_o[c * SH:(c + 1) * SH])
        slot_of_node[c * SH:(c + 1) * SH] = slot
        pi_inv[c, slot] = np.arange(c * SH, (c + 1) * SH)

    own = np.arange(N) // SH
    # chunk-major table position of each node (vectorized _gpos)
    s = slot_of_node
    b = s // P
    jj = np.digitize(b, AG0[1:])
    agoff = np.asarray(AGOFF)[jj]
    agb = np.asarray(AGB)[jj]
    ag0 = np.asarray(AG0)[jj]
    gs = agoff + own * agb * P + (s - ag0 * P)
    src_gs = gs[col]
    src_row = src_gs // 2                  # pair row (fits int16: < 25088)
    src_half = src_gs % 2                  # which half of the 256B row

    cd = row // SH                         # dest core of each edge
    dst_slot = slot_of_node[row]

    cores = []
    maxce = maxco = 0
    for c in range(NCORE):
        m = cd == c
        er = np.stack(
            [dst_slot[m], src_half[m], src_row[m],
             norm[m].view(np.int32).astype(np.int64)], axis=1)
        db = er[:, 0] // P
        er = er[np.lexsort((er[:, 2], er[:, 1], db))]
        db = er[:, 0] // P
        cores.append(er)
        for b in range(NB):
            mb = db == b
            n0 = int((er[mb, 1] == 0).sum())
            n1 = int((er[mb, 1] == 1).sum())
            maxce = max(maxce, -(-n0 // P))
            maxco = max(maxco, -(-n1 // P))
    CE, CO = max(maxce, 1), max(maxco, 1)
    CT = CE + CO
    NT = NB * CT

    def wrap_idx(v):
        n = len(v)
        a = np.zeros((16, n // 16), np.int16)
        a[np.arange(n) % 16, np.arange(n) // 16] = v
        return np.tile(a, (8, 1))

    plans = []
    for c in range(NCORE):
        er = cores[c]
        db = er[:, 0] // P
        nn = NT * P
        idx = np.zeros(nn, np.int64)
        nrm = np.zeros(nn, np.float32)
        dp = np.zeros(nn, np.int64)
        for b in range(NB):
            for h, o in ((0, b * CT * P), (1, (b * CT + CE) * P)):
                mb = (db == b) & (er[:, 1] == h)
                sub = er[mb]
                n = len(sub)
                idx[o:o + n] = sub[:, 2]
                nrm[o:o + n] = sub[:, 3].astype(np.int32).view(np.float32)
                dp[o:o + n] = sub[:, 0] % P
        plans.append({
            "idxs": wrap_idx(idx.astype(np.int16)),
            "nrm": np.ascontiguousarray(nrm.reshape(-1, P).T).astype(bf16),
            "dpt": np.ascontiguousarray(
                dp.reshape(-1, P).T.astype(np.float32)).astype(bf16),
        })

    return dict(plans=plans, pi_inv=pi_inv, CE=CE, CO=CO)


def _build_xt(x, pi_inv):
    xp = np.zeros((TR, F_IN), np.float32)
    for c in range(NCORE):
        valid = pi_inv[c] >= 0
        xp[c * TPC:(c + 1) * TPC][valid] = x[pi_inv[c][valid]]
    return np.ascontiguousarray(xp.T).astype(bf16)  # [96, TR] slot-major (cn, b, p)


# ------------------------------------------------------------------ device
def _build_graph(CE, CO):
    CT = CE + CO
    NT = NB * CT
    NG = -(-NT // G)
    nc = bacc.Bacc("TRN2", target_bir_lowering=False, num_devices=NCORE,
                   num_swdge_queues=NQ)

    f32, b16, i16 = dt.float32, dt.bfloat16, dt.int16
    xt_all = nc.dram_tensor("xt_all", [F_IN, TR], b16, kind="ExternalInput")
    xt_own = nc.dram_tensor("xt_own", [F_IN, TPC], b16, kind="ExternalInput")
    w10 = nc.dram_tensor("w10", [F_IN, F_HID], b16, kind="ExternalInput")
    w11 = nc.dram_tensor("w11", [F_IN, F_HID], b16, kind="ExternalInput")
    w20p = nc.dram_tensor("w20p", [F_HID, FP], b16, kind="ExternalInput")
    w21p = nc.dram_tensor("w21p", [F_HID, FP], b16, kind="ExternalInput")
    b1r = nc.dram_tensor("b1r", [1, F_HID], b16, kind="ExternalInput")
    b2r = nc.dram_tensor("b2r", [1, FP], b16, kind="ExternalInput")
    onesr = nc.dram_tensor("onesr", [1, P], b16, kind="ExternalInput")
    ident = nc.dram_tensor("ident", [P, P], b16, kind="ExternalInput")
    iota = nc.dram_tensor("iota", [P, P], b16, kind="ExternalInput")
    idxs = nc.dram_tensor("idxs", [P, NT * 8], i16, kind="ExternalInput")
    nrm = nc.dram_tensor("nrm", [P, NT], b16, kind="ExternalInput")
    dpt = nc.dram_tensor("dpt", [P, NT], b16, kind="ExternalInput")
    out = nc.dram_tensor("out", [P, NB, F_OUT], f32, kind="ExternalOutput")

    # message tables: [TR, 64] bf16 == [TR//2, 128] pair rows (256B each)
    y1_tab = nc.dram_tensor("y1_tab", [TR, FP], b16, kind="Internal")
    z_bounce = nc.dram_tensor("z_bounce", [TPC, FP], b16, kind="Internal")
    z_full = nc.dram_tensor("z_full", [TR, FP], b16, kind="Internal")

    with tile.TileContext(nc) as tc:
        dma_sems = [nc.alloc_semaphore(f"swdge_dma{q}") for q in range(NQ)]
        qcount = [0] * NQ
        with (
            tc.tile_pool(name="const", bufs=1) as cpool,
            tc.tile_pool(name="persist", bufs=1) as ppool,
            tc.tile_pool(name="hsp", bufs=2) as hsp,
            # gather-message and one-hot pools are shared by both spmm
            # passes: a per-pass scoped pool would reuse the same SBUF stack
            # region, serializing layer 2's first gathers against ALL of
            # layer 1's consumers (~100us hole at the layer boundary).
            tc.tile_pool(name="mgs", bufs=MB) as mgp,
            tc.tile_pool(name="ohps", bufs=3) as ohpp,
            tc.tile_pool(name="psT", bufs=2, space="PSUM") as psT,
            tc.tile_pool(name="psZ", bufs=2, space="PSUM") as psZ,
        ):
            if PREP:
                for s in dma_sems:
                    nc.gpsimd.sem_clear(s)

            # ---- constants / persistent loads
            def load(pool, src, shape, dtype=b16, tag=None):
                t = pool.tile(shape, dtype, tag=tag)
                nc.sync.dma_start(t[:], src[:])
                return t

            w10_t = load(cpool, w10, [F_IN, F_HID], tag="w10")
            w11_t = load(cpool, w11, [F_IN, F_HID], tag="w11")
            w20_t = load(cpool, w20p, [F_HID, FP], tag="w20")
            w21_t = load(cpool, w21p, [F_HID, FP], tag="w21")
            b1_t = load(cpool, b1r, [1, F_HID], tag="b1")
            b2_t = load(cpool, b2r, [1, FP], tag="b2")
            ones_t = load(cpool, onesr, [1, P], tag="ones")
            id_t = load(cpool, ident, [P, P], tag="ident")
            io_t = load(cpool, iota, [P, P], tag="iota")
            ix_t = load(cpool, idxs, [P, NT * 8], i16, tag="ix")
            nrm_t = load(cpool, nrm, [P, NT], tag="nrm")
            dp_t = load(cpool, dpt, [P, NT], tag="dp")
            xo_t = load(ppool, xt_own, [F_IN, TPC], tag="xown")

            hT = ppool.tile([F_HID, TPC], b16, tag="hT")
            z_stages = [
                ppool.tile([P, AGB[j], FP], b16, tag=f"zst{j}", name=f"zst{j}")
                for j in range(len(AGB))
            ]
            out_stage = ppool.tile([P, NB, F_OUT], f32, tag="ost")

            # ---- phase A: y1 = x @ W1_1 for all nodes -> y1_tab (chunk-major)
            with (
                tc.tile_pool(name="xa2", bufs=2) as xa,
                tc.tile_pool(name="ya2", bufs=2) as ya,
                tc.tile_pool(name="psA", bufs=4, space="PSUM") as psA,
            ):
                BPH = min(25, NB)  # blocks per xt slice (25+24)
                for cn in range(NCORE):
                    yst = ya.tile([P, NB, FP], b16, tag="yst")
                    b0 = 0
                    eng = 0
                    for hf, nblk in ((0, BPH), (1, NB - BPH)):
                        if nblk == 0:
                            continue
                        cols = nblk * P
                        xs = xa.tile([F_IN, BPH * P], b16, tag="xs")
                        nc.sync.dma_start(
                            xs[:, :cols],
                            xt_all[:, cn * TPC + b0 * P: cn * TPC + (b0 + nblk) * P])
                        # 8 blocks share one PSUM bank; one batched eviction copy
                        for g0 in range(0, nblk, 8):
                            ng = min(8, nblk - g0)
                            ps = psA.tile([P, 8 * F_HID], f32, tag="psy")
                            for k in range(ng):
                                nc.tensor.matmul(
                                    out=ps[:, k * F_HID:(k + 1) * F_HID],
                                    lhsT=xs[:, (g0 + k) * P:(g0 + k + 1) * P],
                                    rhs=w11_t[:], start=True, stop=True)
                            dst = yst[:, b0 + g0:b0 + g0 + ng, :]
                            src = ps[:, :ng * F_HID].rearrange(
                                "p (k f) -> p k f", k=ng)
                            if eng == 0:
                                nc.vector.tensor_copy(dst, src)
                            else:
                                nc.scalar.copy(dst, src)
                            eng = (eng + 1) % 2
                        b0 += nblk
                    # chunk-major rows: region (j, cn) <- yst blocks [AG0 j..j+1)
                    for j in range(len(AGB)):
                        r0 = AGOFF[j] + cn * AGB[j] * P
                        nc.sync.dma_start(
                            y1_tab[r0:r0 + AGB[j] * P, :].rearrange(
                                "(k p) f -> p k f", p=P),
                            yst[:, AG0[j]:AG0[j + 1], :])

            # ---- spmm pass (shared for both layers)
            def spmm_pass(tab, evict, sfx, post_block=lambda b: None):
                tab_pairs = tab[:].rearrange("(r two) f -> r (two f)", two=2)
                mg, ohp = mgp, ohpp
                with (
                    tc.tile_pool(name="psX" + sfx, bufs=4, space="PSUM") as psX,
                ):
                    g_tiles = [None] * NG

                    def get_group(g):
                        if g_tiles[g] is not None:
                            return g_tiles[g]
                        ncg = min(G, NT - g * G)
                        ni = ncg * P
                        q = g % NQ
                        m = mg.tile([P, G, 2 * FP], b16, tag="mg")
                        if PREP:
                            nc.gpsimd.dma_gather(
                                m[:, :ncg, :], tab_pairs,
                                ix_t[:, g * G * 8:(g * G + ncg) * 8],
                                ni, ni, 2 * FP, single_packet=False,
                                queue_num=q, prepare_only=True,
                                sem=dma_sems[q])
                            nc.gpsimd.trigger_dma(count=None, queue_num=q)
                            qcount[q] += 1
                            nc.vector.wait_ge(dma_sems[q], 16 * qcount[q])
                        else:
                            nc.gpsimd.dma_gather(
                                m[:, :ncg, :], tab_pairs,
                                ix_t[:, g * G * 8:(g * G + ncg) * 8],
                                ni, ni, 2 * FP, single_packet=False,
                                queue_num=q)
                        # edge norms are folded into the one-hot instead of
                        # scaling m here: matmuls then consume the gathered
                        # tile directly (no DVE op on the DMA->PE path)
                        g_tiles[g] = m
                        return m

                    for b in range(NB):
                        ps = psX.tile([P, FP], f32, tag="acc")
                        oh = ohp.tile([P, CT * P], b16, tag="oh")
                        nc.vector.tensor_tensor(
                            out=oh[:].rearrange("p (c j) -> p c j", c=CT),
                            in0=dp_t[:, b * CT:(b + 1) * CT].to_broadcast(
                                [P, CT, P]),
                            in1=bass.AP(io_t[:].tensor, io_t[:].offset,
                                        [io_t[:].ap[0], [0, CT], [1, P]]),
                            op=mybir.AluOpType.is_equal)
                        nc.vector.tensor_tensor(
                            out=oh[:].rearrange("p (c j) -> p c j", c=CT),
                            in0=oh[:].rearrange("p (c j) -> p c j", c=CT),
                            in1=nrm_t[:, b * CT:(b + 1) * CT].to_broadcast(
                                [P, CT, P]),
                            op=mybir.AluOpType.mult)
                        for j in range(CT):
                            q = b * CT + j
                            m = get_group(q // G)
                            h = 0 if j < CE else 1
                            nc.tensor.matmul(
                                out=ps[:], lhsT=oh[:, j * P:(j + 1) * P],
                                rhs=m[:, q % G, h * FP:(h + 1) * FP],
                                start=(j == 0), stop=False)
                        evict(b, ps)
                        post_block(b)

            # ---- layer 1 eviction: h block
            def evict_l1(b, ps):
                nc.tensor.matmul(out=ps[:], lhsT=xo_t[:, b * P:(b + 1) * P],
                                 rhs=w10_t[:], start=False, stop=False)
                nc.tensor.matmul(out=ps[:], lhsT=ones_t[:], rhs=b1_t[:],
                                 start=False, stop=True)
                hs = hsp.tile([P, F_HID], b16, tag="hs")
                nc.scalar.activation(hs[:], ps[:], mybir.ActivationFunctionType.Relu)
                pt = psT.tile([F_HID, P], b16, tag="pt")
                nc.tensor.transpose(out=pt[:], in_=hs[:], identity=id_t[:])
                nc.vector.tensor_copy(hT[:, b * P:(b + 1) * P], pt[:])
                pz = psZ.tile([P, FP], f32, tag="pz")
                nc.tensor.matmul(out=pz[:], lhsT=hT[:, b * P:(b + 1) * P],
                                 rhs=w21_t[:], start=True, stop=True)
                j = _chunk_of_block(b)
                nc.scalar.copy(z_stages[j][:, b - AG0[j], :], pz[:])

            # Stage each z chunk to HBM as soon as its last block is evicted,
            # but delay the AllGather *emission* ~10 blocks: the collective is
            # issued on the in-order Pool engine, and Pool runs ~14 blocks of
            # gather issue ahead of the evictions — emitting the AG at the
            # data-ready block stalls the whole gather chain behind its wait.
            AGDELAY = 10

            def post_l1(b):
                for j in range(len(AGB)):
                    if b == AG0[j + 1] - 1:
                        r0 = AG0[j] * P
                        nc.sync.dma_start(
                            z_bounce[r0:r0 + AGB[j] * P, :].rearrange(
                                "(k p) f -> p k f", p=P),
                            z_stages[j][:])
                    if b == min(AG0[j + 1] - 1 + AGDELAY, NB - 1):
                        r0 = AG0[j] * P
                        nc.gpsimd.collective_compute(
                            "AllGather", mybir.AluOpType.bypass,
                            replica_groups=[list(range(NCORE))],
                            ins=[z_bounce[r0:r0 + AGB[j] * P, :].opt()],
                            outs=[z_full[AGOFF[j]:
                                         AGOFF[j] + 8 * AGB[j] * P, :].opt()],
                        )

            spmm_pass(y1_tab, evict_l1, "a", post_l1)

            # ---- layer 2 eviction: out block
            def evict_l2(b, ps):
                nc.tensor.matmul(out=ps[:], lhsT=hT[:, b * P:(b + 1) * P],
                                 rhs=w20_t[:], start=False, stop=False)
                nc.tensor.matmul(out=ps[:], lhsT=ones_t[:], rhs=b2_t[:],
                                 start=False, stop=True)
                if b % 2 == 0:
                    nc.scalar.copy(out_stage[:, b, :], ps[:, :F_OUT])
                else:
                    nc.vector.tensor_copy(out_stage[:, b, :], ps[:, :F_OUT])

            spmm_pass(z_full, evict_l2, "b")

            nc.sync.dma_start(out[:], out_stage[:])

    nc.compile()
    return nc


_GRAPH_CACHE = {}


def kernel(x, edge_index, W1_0, W1_1, b1, W2_0, W2_1, b2):
    x = np.asarray(x, np.float32)
    W1_0 = np.asarray(W1_0, np.float32)
    W1_1 = np.asarray(W1_1, np.float32)
    b1 = np.asarray(b1, np.float32)
    W2_0 = np.asarray(W2_0, np.float32)
    W2_1 = np.asarray(W2_1, np.float32)
    b2 = np.asarray(b2, np.float32)

    plan = _build_plan(edge_index)
    CE, CO = plan["CE"], plan["CO"]

    xt = _build_xt(x, plan["pi_inv"])
    w20p = np.zeros((F_HID, FP), np.float32); w20p[:, :F_OUT] = W2_0
    w21p = np.zeros((F_HID, FP), np.float32); w21p[:, :F_OUT] = W2_1
    b2p = np.zeros((1, FP), np.float32); b2p[0, :F_OUT] = b2
    ident = np.eye(P, dtype=np.float32)
    iota = np.tile(np.arange(P, dtype=np.float32), (P, 1))
    ones = np.ones((1, P), np.float32)

    common = dict(
        xt_all=xt,
        w10=W1_0.astype(bf16), w11=W1_1.astype(bf16),
        w20p=w20p.astype(bf16), w21p=w21p.astype(bf16),
        b1r=b1.reshape(1, F_HID).astype(bf16), b2r=b2p.astype(bf16),
        onesr=ones.astype(bf16), ident=ident.astype(bf16),
        iota=iota.astype(bf16),
    )
    in_maps = []
    for c in range(NCORE):
        pl = plan["plans"][c]
        m = dict(common)
        m["xt_own"] = np.ascontiguousarray(xt[:, c * TPC:(c + 1) * TPC])
        m["idxs"] = pl["idxs"]
        m["nrm"] = pl["nrm"]
        m["dpt"] = pl["dpt"]
        in_maps.append(m)

    global G, NQ, PREP
    res = None
    last_exc = None
    for g_try, nq_try, prep_try in (
            (16, 4, False), (16, 2, False), (4, 1, False)):
        G, NQ, PREP = g_try, nq_try, prep_try
        key = (CE, CO, g_try, nq_try, prep_try)
        try:
            if key not in _GRAPH_CACHE:
                _GRAPH_CACHE[key] = _build_graph(CE, CO)
            res = run_bass_kernel_spmd(
                _GRAPH_CACHE[key], in_maps, core_ids=list(range(NCORE)))
            break
        except Exception as e:  # noqa: BLE001 - retry with safer gather size
            last_exc = e
            import time as _t
            _t.sleep(10)
    if res is None:
        raise last_exc
    kernel.last_result = res

    out_full = np.zeros((N, F_OUT), np.float32)
    pi_inv = plan["pi_inv"]
    for c in range(NCORE):
        o = res.results[c]["out"].transpose(1, 0, 2).reshape(TPC, F_OUT)
        valid = pi_inv[c] >= 0
        out_full[pi_inv[c][valid]] = o[valid]
    return out_full
